# revision 5
# baseline (speedup 1.0000x reference)
"""AUGRU (attention-modulated GRU, DIEN InterestEvolve) Trainium2 Bass kernel.

Sharding: data-parallel over batch across 8 NeuronCores (256 rows/core).
Layout on device: feature-major [U=128 partitions, batch free], so the
recurrent matmuls consume h directly (lhsT = W chunk [K=128, M=128],
rhs = hT/xT [K=128, N=batch]).

Per core, per timestep t (two independent 128-wide batch chunks pipelined):
    z_u = Wu_h.T @ hT + Wu_x.T @ xT     (PSUM, 2 accumulating matmuls)
    z_r = Wr_h.T @ hT + Wr_x.T @ xT
    ur  = sigmoid(z_ur + b)             (one fused ACT op when bu == br)
    rh  = r * h                         (DVE, bf16)
    z_h = Wh_x.T @ xT + Wh_rh.T @ rhT
    hh  = tanh(z_h + bh)                (ACT)
    ua  = u * a_bcast                   (GpSimd, off critical path)
    d   = hh - h; p = ua * d; h' = h+p  (DVE)

x is host-pretransposed to (T, D, Bc) bf16 and streamed; a is broadcast
across partitions by a stride-0 DMA from (T, 1, Bc) bf16.
"""

import sys
import types

import numpy as np

# --- runtime shims (standalone container) --------------------------------- #


def _install_shims():
    import concourse.mybir as mybir
    import concourse.tile as tile_mod
    from concourse.vector_clock import ScopedClock

    if "antenv.axon_hooks" not in sys.modules:
        m = types.ModuleType("antenv.axon_hooks")
        m._hook = None
        m.set_axon_ntff_profile_hook = lambda h: setattr(m, "_hook", h)
        m.get_axon_ntff_profile_hook = lambda: m._hook
        sys.modules["antenv.axon_hooks"] = m
        import antenv

        antenv.axon_hooks = m
        try:
            from trn_agent_boot.trn_boot import _ntff_profile_via_ctypes

            m.set_axon_ntff_profile_hook(
                _ntff_profile_via_ctypes("/opt/axon/libaxon_pjrt.so")
            )
        except Exception:
            pass

    # walrus in this image allows only one semaphore wait per TPB_CTRL
    # (Drain) instruction; split the tail drain's waits across extra drains.
    def _drain_and_barrier(self, tick_clock, wait_clock):
        drain_inst = self.nc.sync.drain()
        wait_clock.add_sem_waits(
            drain_inst.ins, ScopedClock({None: tick_clock.global_clock})
        )
        si = drain_inst.ins.sync_info
        if si is not None and len(si.on_wait) > 1:
            waits = list(si.on_wait)
            drain_inst.ins.sync_info = mybir.SyncInfo(
                on_update=list(si.on_update), on_wait=waits[:1]
            )
            for k in range(1, len(waits)):
                extra = self.nc.sync.drain()
                extra.ins.sync_info = mybir.SyncInfo(
                    on_update=[], on_wait=waits[k : k + 1]
                )
        self.nc.all_engine_barrier()
        assert self.sems is not None
        popped = self.nc._tile_sem_poison_stack.pop()
        assert popped is self._sem_poison
        self.nc.clear_and_free_semaphores(list(self.sems.allocated().values()))
        self.nc.all_engine_barrier()

    tile_mod.TileContext._drain_and_barrier = _drain_and_barrier


_install_shims()

import concourse.bass as bass  # noqa: E402
import concourse.mybir as mybir  # noqa: E402
import concourse.tile as tile  # noqa: E402

B, T, D, U = 2048, 200, 128, 128
NCORES = 8
BC = B // NCORES  # batch per core
CH = 2  # pipelined batch chunks per core
C = BC // CH  # chunk width (free dim of every op)

_cache = {}


def _build_nc(fused_ur: bool):
    nc = bass.Bass()
    f32, bf16 = mybir.dt.float32, mybir.dt.bfloat16

    xT = nc.dram_tensor("xT", [T, D, BC], bf16, kind="ExternalInput")
    aRow = nc.dram_tensor("aRow", [T, 1, BC], bf16, kind="ExternalInput")
    w = nc.dram_tensor("w", [6, 128, 128], bf16, kind="ExternalInput")
    bia = nc.dram_tensor("bia", [3, 128, 1], f32, kind="ExternalInput")
    out = nc.dram_tensor("out", [128, BC], bf16, kind="ExternalOutput")

    SIG = mybir.ActivationFunctionType.Sigmoid
    TANH = mybir.ActivationFunctionType.Tanh

    with tile.TileContext(nc) as tc:
        with (
            tc.tile_pool(name="singles", bufs=1) as singles,
            tc.tile_pool(name="xs", bufs=6) as xs_pool,
            tc.tile_pool(name="as_", bufs=6) as as_pool,
            tc.tile_pool(name="hs", bufs=3) as hs_pool,
            tc.tile_pool(name="gates", bufs=3) as gates_pool,
            tc.tile_pool(name="ps", bufs=2, space="PSUM") as ps_pool,
        ):
            # weights: [128, 6*128] bf16; slices are lhsT tiles [K=128, M=128]
            wsb = singles.tile([128, 6, 128], bf16)
            nc.sync.dma_start(out=wsb[:], in_=w.rearrange("s k m -> k s m"))
            w_uh, w_ux = wsb[:, 0, :], wsb[:, 1, :]
            w_rh, w_rx = wsb[:, 2, :], wsb[:, 3, :]
            w_hx, w_hrh = wsb[:, 4, :], wsb[:, 5, :]

            bsb = singles.tile([128, 3], f32)
            nc.sync.dma_start(out=bsb[:], in_=bia.rearrange("s p one -> p (s one)"))
            b_u, b_r, b_h = bsb[:, 0:1], bsb[:, 1:2], bsb[:, 2:3]

            # initial state h0 = 0
            hcur = []
            for c in range(CH):
                h0 = hs_pool.tile([128, C], bf16, tag=f"h{c}")
                nc.vector.memset(h0, 0.0)
                hcur.append(h0)

            for t in range(T):
                x_t = xs_pool.tile([128, BC], bf16, tag="x")
                nc.sync.dma_start(out=x_t[:], in_=xT[t])
                a_t = as_pool.tile([128, BC], bf16, tag="a")
                a_slice = aRow[t]
                a_bcast = bass.AP(
                    tensor=a_slice.tensor,
                    offset=a_slice.offset,
                    ap=[[0, 128], [1, BC]],
                )
                nc.sync.dma_start(out=a_t[:], in_=a_bcast)

                for c in range(CH):
                    h = hcur[c]
                    xc = x_t[:, c * C : (c + 1) * C]
                    ac = a_t[:, c * C : (c + 1) * C]

                    ps = ps_pool.tile([128, 3 * C], f32, tag=f"ps{c}")
                    zu, zr, zh = ps[:, 0:C], ps[:, C : 2 * C], ps[:, 2 * C : 3 * C]
                    nc.tensor.matmul(zu, w_uh, h, start=True, stop=False)
                    nc.tensor.matmul(zu, w_ux, xc, start=False, stop=True)
                    nc.tensor.matmul(zr, w_rh, h, start=True, stop=False)
                    nc.tensor.matmul(zr, w_rx, xc, start=False, stop=True)
                    nc.tensor.matmul(zh, w_hx, xc, start=True, stop=False)

                    ur = gates_pool.tile([128, 2 * C], bf16, tag=f"ur{c}")
                    if fused_ur:
                        nc.scalar.activation(ur, ps[:, 0 : 2 * C], SIG, bias=b_u)
                    else:
                        nc.scalar.activation(ur[:, 0:C], zu, SIG, bias=b_u)
                        nc.scalar.activation(ur[:, C : 2 * C], zr, SIG, bias=b_r)
                    u_g, r_g = ur[:, 0:C], ur[:, C : 2 * C]

                    rh = gates_pool.tile([128, C], bf16, tag=f"rh{c}")
                    nc.vector.tensor_mul(rh, r_g, h)
                    nc.tensor.matmul(zh, w_hrh, rh, start=False, stop=True)

                    hh = gates_pool.tile([128, C], bf16, tag=f"hh{c}")
                    nc.scalar.activation(hh, zh, TANH, bias=b_h)

                    ua = gates_pool.tile([128, C], bf16, tag=f"ua{c}")
                    nc.gpsimd.tensor_mul(ua, u_g, ac)

                    d = gates_pool.tile([128, C], bf16, tag=f"d{c}")
                    nc.vector.tensor_sub(d, hh, h)
                    p = gates_pool.tile([128, C], bf16, tag=f"p{c}")
                    nc.vector.tensor_mul(p, ua, d)
                    hn = hs_pool.tile([128, C], bf16, tag=f"h{c}")
                    nc.vector.tensor_add(hn, h, p)
                    hcur[c] = hn

            for c in range(CH):
                nc.sync.dma_start(out=out[:, c * C : (c + 1) * C], in_=hcur[c])

    _split_multiwaits(nc)
    return nc


def _split_multiwaits(nc):
    """walrus in this image accepts one semaphore wait per instruction;
    hoist extra waits onto same-engine NOPs inserted just before."""
    for fn in nc.m.functions:
        for b in fn.blocks:
            insts = b.instructions
            if not any(
                i.sync_info is not None and len(i.sync_info.on_wait) > 1
                for i in insts
            ):
                continue
            out_list = []
            for inst in insts:
                si = inst.sync_info
                if si is not None and len(si.on_wait) > 1:
                    waits = list(si.on_wait)
                    for k, w in enumerate(waits[1:]):
                        out_list.append(
                            mybir.InstNoOp(
                                name=f"{inst.name}_w{k}",
                                engine=inst.engine,
                                bass_nofuse=True,
                                sync_info=mybir.SyncInfo(
                                    on_wait=[w], on_update=[]
                                ),
                            )
                        )
                    inst.sync_info = mybir.SyncInfo(
                        on_wait=waits[:1], on_update=list(si.on_update)
                    )
                out_list.append(inst)
            b.instructions = out_list


def kernel(history_state, attention_score, Wu, bu, Wr, br, Wh, bh):
    from concourse.bass_utils import run_bass_kernel_spmd

    history_state = np.asarray(history_state)
    attention_score = np.asarray(attention_score)
    Wu, bu = np.asarray(Wu), np.asarray(bu)
    Wr, br = np.asarray(Wr), np.asarray(br)
    Wh, bh = np.asarray(Wh), np.asarray(bh)

    fused_ur = bool(np.array_equal(bu, br))
    key = ("nc", fused_ur)
    if key not in _cache:
        _cache[key] = _build_nc(fused_ur)
    nc = _cache[key]

    import ml_dtypes

    w_all = np.stack(
        [Wu[:U], Wu[U:], Wr[:U], Wr[U:], Wh[:D], Wh[D:]], axis=0
    ).astype(ml_dtypes.bfloat16)
    bias_all = np.stack([bu, br, bh], axis=0).reshape(3, 128, 1).astype(np.float32)

    in_maps = []
    for i in range(NCORES):
        sl = slice(i * BC, (i + 1) * BC)
        hs = history_state[sl]  # (BC, T, D)
        xT = np.ascontiguousarray(hs.transpose(1, 2, 0)).astype(ml_dtypes.bfloat16)
        a = attention_score[sl, :, 0]  # (BC, T)
        aRow = np.ascontiguousarray(a.T).reshape(T, 1, BC).astype(ml_dtypes.bfloat16)
        in_maps.append({"xT": xT, "aRow": aRow, "w": w_all, "bia": bias_all})

    global _last_in_maps
    _last_in_maps = in_maps

    res = run_bass_kernel_spmd(nc, in_maps, core_ids=list(range(NCORES)))
    outs = [r["out"] for r in res.results]  # each [128, BC] bf16
    h = np.concatenate([o.T for o in outs], axis=0)  # (B, U)
    return h.astype(history_state.dtype)
